# revision 2
# baseline (speedup 1.0000x reference)
"""DBRX MoE experts kernel for 8 Trainium2 NeuronCores (expert-parallel, v2).

Strategy:
  - Host: router matmul + softmax + top-4 + renormalize (tiny), gather tokens
    per expert, sort experts by token count and pair largest-with-smallest so
    the two per-core expert slots are (C0, C1) with C0+C1 near the mean,
    pre-transpose/re-tile all operands, cast weights + activations to bf16.
  - Device (SPMD, 8 cores, 2 experts each): per expert, SwiGLU FFN over its
    gathered tokens, everything transposed ([feature, token]) so the chain
    needs no on-chip transposes:
       GT[i,c] = W1T.T@XgT, UT[i,c] = V1T.T@XgT  (accumulate over d)
       HT[i,c] = silu(GT)*UT                      (ACT + DVE, bf16)
       YT[d,c] = W2T.T@HT                         (accumulate over all 32 i-tiles)
    PSUM budget: 6 banks for the g/u accumulators (3 token-chunks each),
    2 banks for y — stage-disjoint tags so the down-projection of one expert
    overlaps the up-projection of the next.
  - Host: scale rows by gates and scatter-add into the output.
"""
import sys
sys.path.insert(0, "/opt/trn_rl_repo")
import numpy as np
import ml_dtypes

import concourse.bass as bass
import concourse.mybir as mybir
import concourse.tile as tile
import concourse.tile_sem_assignment as _tsa

# This walrus build only supports ONE sync-wait command per instruction.
# Route all HWDGE DMA completions through a single semaphore so consumers
# need at most one DMA wait...
_tsa.NUM_HWDGE_SEMS = 1

N_CORES = 8
E = 16
E_LOC = 2
D = 2048
I = 4096
TOP_K = 4
NDT = D // 128   # 16 d-tiles
NIT = I // 128   # 32 i-tiles

_BF16 = mybir.dt.bfloat16
_F32 = mybir.dt.float32
_NPBF16 = ml_dtypes.bfloat16


def _split_multi_waits(nc):
    """...and split any instruction that still carries >1 sync wait into
    single-wait EventSemaphore prefixes (semantically identical: waits are
    ANDed, the sequencer executes them in order before the instruction)."""
    ctr = 0
    for f in nc.m.functions:
        for blk in f.blocks:
            insts = list(blk.instructions)
            out = []
            changed = False
            for inst in insts:
                si = inst.sync_info
                if si is not None and si.on_wait is not None and len(si.on_wait) > 1:
                    waits = list(si.on_wait)
                    for w in waits[:-1]:
                        ctr += 1
                        out.append(mybir.InstEventSemaphore(
                            name=f"wsplit_{ctr}",
                            engine=inst.engine,
                            ins=[], outs=[],
                            sync_info=mybir.SyncInfo(on_wait=[w], on_update=[]),
                            bass_nofuse=True,
                        ))
                    inst.sync_info = mybir.SyncInfo(
                        on_wait=[waits[-1]], on_update=list(si.on_update or []))
                    changed = True
                out.append(inst)
            if changed:
                blk.instructions.clear()
                for i2 in out:
                    blk.add_instruction(i2)


def _chunks(n):
    """Split even-length [0, n) into even-sized PSUM-bank chunks (<=512 each)."""
    assert n % 2 == 0
    if n <= 512:
        return [(0, n)]
    k = -(-n // 512)
    sizes = [(n // k) & ~1] * k
    rem, j = n - sum(sizes), 0
    while rem > 0:
        sizes[j] += 2
        rem -= 2
        j = (j + 1) % k
    out, s = [], 0
    for sz in sizes:
        out.append((s, sz))
        s += sz
    return out


def _build_nc(C0, C1, rep=1, internal_out=False):
    """One SPMD program; per-core inputs differ only in data.

    Two expert slots per core with token capacities C0 >= C1 (experts are
    paired big-with-small on the host). Per expert: all 32 i-tiles of HT
    stay resident in SBUF (bf16), so the down-projection runs once per
    expert with no DRAM accumulation.
    """
    nc = bass.Bass(target_bir_lowering=False)
    xt_d = [
        nc.dram_tensor("xt0", [NDT, 128, C0], _BF16, kind="ExternalInput"),
        nc.dram_tensor("xt1", [NDT, 128, C1], _BF16, kind="ExternalInput"),
    ]
    wv1_d = nc.dram_tensor("wv1", [E_LOC, NIT, 128, 2 * NDT * 128], _BF16,
                           kind="ExternalInput")
    w2_d = nc.dram_tensor("w2", [E_LOC, NDT, 128, NIT * 128], _BF16,
                          kind="ExternalInput")
    okind = "Internal" if internal_out else "ExternalOutput"
    yt_d = [
        nc.dram_tensor("yt0", [NDT, 128, C0], _F32, kind=okind),
        nc.dram_tensor("yt1", [NDT, 128, C1], _F32, kind=okind),
    ]
    sync_d = (nc.dram_tensor("sync", [128, 2], _F32, kind="ExternalOutput")
              if internal_out else None)

    with tile.TileContext(nc) as tc:
        with (
            tc.tile_pool(name="xt", bufs=2) as xt_pool,
            tc.tile_pool(name="ht", bufs=1) as ht_pool,
            tc.tile_pool(name="wv", bufs=2) as wv_pool,
            tc.tile_pool(name="w2", bufs=2) as w2_pool,
            tc.tile_pool(name="hs", bufs=2) as hs_pool,
            tc.tile_pool(name="yo", bufs=3) as yo_pool,
            tc.tile_pool(name="psg", bufs=1, space="PSUM") as psg,
            tc.tile_pool(name="psy", bufs=2, space="PSUM") as psy,
        ):
            pools = (xt_pool, ht_pool, wv_pool, w2_pool, hs_pool, yo_pool,
                     psg, psy)
            for rp in range(rep):
                for el, C in enumerate((C0, C1)):
                    _emit_expert(nc, pools, xt_d[el], wv1_d, w2_d, yt_d[el],
                                 rp, el, C)
            if sync_d is not None:
                sy = yo_pool.tile([128, 2], _F32, tag="sync", name="sync_t")
                nc.vector.memset(sy[:], 0.0)
                nc.sync.dma_start(sync_d[:, :], sy[:])
    nc.finalize()
    _split_multi_waits(nc)
    return nc


def _emit_expert(nc, pools, xt_d, wv1_d, w2_d, yt_d, rp, el, C):
    (xt_pool, ht_pool, wv_pool, w2_pool, hs_pool, yo_pool, psg, psy) = pools
    ch = _chunks(C)
    tb = f"{rp}_{el}"
    # --- token tiles (resident for the whole expert; prefetched) ---
    xts = xt_pool.tile([128, NDT, C], _BF16, tag="xt", name=f"xt_{tb}")
    for dt in range(NDT):
        nc.sync.dma_start(xts[:, dt, :], xt_d[dt, :, :])
    hts = ht_pool.tile([128, NIT, C], _BF16, tag="ht", name=f"ht_{tb}")
    # --- stage 1+2: HT[it] = silu(W1T.T@X) * (V1T.T@X), all 32 i-tiles ---
    for it in range(NIT):
        wv = wv_pool.tile([128, 2, NDT, 128], _BF16, tag="wv",
                          name=f"wv_{tb}_{it}")
        nc.sync.dma_start(
            wv[:], wv1_d[el, it].rearrange("p (w t i) -> p w t i",
                                           w=2, t=NDT))
        gs = [psg.tile([128, cn], _F32, tag=f"g{ci}", name=f"g{ci}_{tb}_{it}")
              for ci, (c0, cn) in enumerate(ch)]
        for dt in range(NDT):
            for ci, (c0, cn) in enumerate(ch):
                nc.tensor.matmul(
                    gs[ci][:], wv[:, 0, dt, :], xts[:, dt, c0:c0 + cn],
                    start=(dt == 0), stop=(dt == NDT - 1))
        hss = []
        for ci, (c0, cn) in enumerate(ch):
            h1 = hs_pool.tile([128, cn], _BF16, tag=f"hs{ci}",
                              name=f"hs{ci}_{tb}_{it}")
            nc.scalar.activation(h1[:], gs[ci][:],
                                 mybir.ActivationFunctionType.Silu)
            hss.append(h1)
        us = [psg.tile([128, cn], _F32, tag=f"u{ci}", name=f"u{ci}_{tb}_{it}")
              for ci, (c0, cn) in enumerate(ch)]
        for dt in range(NDT):
            for ci, (c0, cn) in enumerate(ch):
                nc.tensor.matmul(
                    us[ci][:], wv[:, 1, dt, :], xts[:, dt, c0:c0 + cn],
                    start=(dt == 0), stop=(dt == NDT - 1))
        for ci, (c0, cn) in enumerate(ch):
            nc.vector.tensor_tensor(
                out=hts[:, it, c0:c0 + cn], in0=us[ci][:], in1=hss[ci][:],
                op=mybir.AluOpType.mult)
    # --- stage 3: YT[dt] = W2T.T @ HT over all 32 i-tiles ---
    w2_re = w2_d[el].rearrange("t p (u i) -> t p u i", u=NIT)
    for dt in range(NDT):
        w2t = w2_pool.tile([128, NIT, 128], _BF16, tag="w2",
                           name=f"w2_{tb}_{dt}")
        # w2 loads ride the ACT HWDGE ring so they never queue behind the
        # (pool-throttled) wv stream on the SP ring at stage boundaries.
        nc.scalar.dma_start(w2t[:], w2_re[dt])
        for ci, (c0, cn) in enumerate(ch):
            yt = psy.tile([128, cn], _F32, tag="y", name=f"y{ci}_{tb}_{dt}")
            for itl in range(NIT):
                nc.tensor.matmul(
                    yt[:], w2t[:, itl, :], hts[:, itl, c0:c0 + cn],
                    start=(itl == 0), stop=(itl == NIT - 1))
            yo = yo_pool.tile([128, cn], _F32, tag="yo",
                              name=f"yo{ci}_{tb}_{dt}")
            nc.scalar.activation(yo[:], yt[:],
                                 mybir.ActivationFunctionType.Copy)
            # y writes go out on the SWDGE ring, off both weight streams.
            nc.gpsimd.dma_start(yt_d[dt, :, c0:c0 + cn], yo[:])


def _prepare(hidden_states, router_w, ws, w2s, rep=1, internal_out=False):
    hs = np.ascontiguousarray(hidden_states, dtype=np.float32)
    rw = np.ascontiguousarray(router_w, dtype=np.float32)
    ws = np.asarray(ws, dtype=np.float32)
    w2s = np.asarray(w2s, dtype=np.float32)
    T, D_ = hs.shape
    assert (D_, ws.shape[0], ws.shape[1], w2s.shape[1], w2s.shape[2]) == \
        (D, E, 2 * I, D, I), "kernel compiled for DBRX 16x(2048->4096) shapes"

    # ---- routing on host (softmax -> top-4 -> renormalize) ----
    logits = hs @ rw.T                                   # [T, E]
    m = logits.max(axis=-1, keepdims=True)
    p = np.exp(logits - m)
    p /= p.sum(axis=-1, keepdims=True)
    topk_idx = np.argpartition(-p, TOP_K - 1, axis=-1)[:, :TOP_K]   # [T, 4]
    topk_val = np.take_along_axis(p, topk_idx, axis=-1)
    gates_w = topk_val / topk_val.sum(axis=-1, keepdims=True)

    tok_idx, tok_gate = [None] * E, [None] * E
    flat_e = topk_idx.ravel()
    flat_g = gates_w.ravel()
    flat_t = np.repeat(np.arange(T), TOP_K)
    order = np.argsort(flat_e, kind="stable")
    se, st, sg = flat_e[order], flat_t[order], flat_g[order]
    bounds = np.searchsorted(se, np.arange(E + 1))
    for e in range(E):
        tok_idx[e] = st[bounds[e]:bounds[e + 1]]
        tok_gate[e] = sg[bounds[e]:bounds[e + 1]]

    # ---- pair experts big-with-small across cores ----
    counts = np.array([len(t) for t in tok_idx])
    desc = np.argsort(-counts, kind="stable")
    # core c runs experts (desc[c], desc[15-c]); slot capacities:
    slot_expert = [[int(desc[c]), int(desc[2 * N_CORES - 1 - c])]
                   for c in range(N_CORES)]
    Cs = []
    for sl in range(E_LOC):
        Cmax = max(4, int(max(counts[slot_expert[c][sl]]
                              for c in range(N_CORES))))
        Cs.append(Cmax + (Cmax % 2))
    C0, C1 = Cs

    hs_bf = hs.astype(_NPBF16)
    ws_bf = ws.astype(_NPBF16)
    w2s_bf = w2s.astype(_NPBF16)

    # ---- build per-core inputs ----
    in_maps = []
    for c in range(N_CORES):
        im = {}
        wv1 = np.empty((E_LOC, NIT, 128, 2 * NDT * 128), dtype=_NPBF16)
        w2 = np.empty((E_LOC, NDT, 128, NIT * 128), dtype=_NPBF16)
        for el, Ce in enumerate((C0, C1)):
            e = slot_expert[c][el]
            xt = np.zeros((NDT, 128, Ce), dtype=_NPBF16)
            xg = hs_bf[tok_idx[e]]                       # [n_e, D]
            # xt[dt, p, c] = xg[c, dt*128+p]
            xt[:, :, :len(tok_idx[e])] = xg.T.reshape(NDT, 128, -1)
            im[f"xt{el}"] = xt
            w1 = ws_bf[e, :I, :]                         # [I, D]
            v1 = ws_bf[e, I:, :]
            # wv1[el, it, p, (w, dt, ii)] = {w1,v1}[it*128+ii, dt*128+p]
            wv = np.stack([w1, v1]).reshape(2, NIT, 128, NDT, 128)
            wv = wv.transpose(1, 4, 0, 3, 2)             # [it, p, w, dt, ii]
            wv1[el] = np.ascontiguousarray(wv).reshape(NIT, 128, -1)
            # w2[el, dt, p, (it, ii)] = w2s[e, dt*128+ii, it*128+p]
            w2e = w2s_bf[e].reshape(NDT, 128, NIT, 128)  # [dt, ii, it, p]
            w2[el] = np.ascontiguousarray(
                w2e.transpose(0, 3, 2, 1)).reshape(NDT, 128, -1)
        im["wv1"] = wv1
        im["w2"] = w2
        in_maps.append(im)

    def combine(results):
        out = np.zeros((T, D), dtype=np.float32)
        for c in range(N_CORES):
            for el in range(E_LOC):
                e = slot_expert[c][el]
                n_e = len(tok_idx[e])
                if n_e == 0:
                    continue
                yt = results[c][f"yt{el}"]               # [NDT, 128, C]
                y = yt.reshape(D, -1)[:, :n_e].T         # [n_e, D]
                out[tok_idx[e]] += tok_gate[e][:, None].astype(np.float32) * y
        return out

    nc = _build_nc(C0, C1, rep=rep, internal_out=internal_out)
    return {"nc": nc, "in_maps": in_maps, "combine": combine, "C": (C0, C1)}


def kernel(hidden_states, router_w, ws, w2s):
    from concourse.bass_utils import run_bass_kernel_spmd
    prep = _prepare(hidden_states, router_w, ws, w2s)
    res = run_bass_kernel_spmd(prep["nc"], prep["in_maps"],
                               core_ids=list(range(N_CORES)))
    return prep["combine"](res.results)


# revision 3
# speedup vs baseline: 1.0321x; 1.0321x over previous
"""DBRX MoE experts kernel for 8 Trainium2 NeuronCores (expert-parallel, v2).

Strategy:
  - Host: router matmul + softmax + top-4 + renormalize (tiny), gather tokens
    per expert, sort experts by token count and pair largest-with-smallest so
    the two per-core expert slots are (C0, C1) with C0+C1 near the mean,
    pre-transpose/re-tile all operands, cast weights + activations to bf16.
  - Device (SPMD, 8 cores, 2 experts each): per expert, SwiGLU FFN over its
    gathered tokens, everything transposed ([feature, token]) so the chain
    needs no on-chip transposes:
       GT[i,c] = W1T.T@XgT, UT[i,c] = V1T.T@XgT  (accumulate over d)
       HT[i,c] = silu(GT)*UT                      (ACT + DVE, bf16)
       YT[d,c] = W2T.T@HT                         (accumulate over all 32 i-tiles)
    PSUM budget: 6 banks for the g/u accumulators (3 token-chunks each),
    2 banks for y — stage-disjoint tags so the down-projection of one expert
    overlaps the up-projection of the next.
  - Host: scale rows by gates and scatter-add into the output.
"""
import sys
sys.path.insert(0, "/opt/trn_rl_repo")
import numpy as np
import ml_dtypes

import concourse.bass as bass
import concourse.mybir as mybir
import concourse.tile as tile
import concourse.tile_sem_assignment as _tsa

# This walrus build only supports ONE sync-wait command per instruction.
# Route all HWDGE DMA completions through a single semaphore so consumers
# need at most one DMA wait...
_tsa.NUM_HWDGE_SEMS = 1

N_CORES = 8
E = 16
E_LOC = 2
D = 2048
I = 4096
TOP_K = 4
NDT = D // 128   # 16 d-tiles
NIT = I // 128   # 32 i-tiles

_BF16 = mybir.dt.bfloat16
_F32 = mybir.dt.float32
_NPBF16 = ml_dtypes.bfloat16


def _split_multi_waits(nc):
    """...and split any instruction that still carries >1 sync wait into
    single-wait EventSemaphore prefixes (semantically identical: waits are
    ANDed, the sequencer executes them in order before the instruction)."""
    ctr = 0
    for f in nc.m.functions:
        for blk in f.blocks:
            insts = list(blk.instructions)
            out = []
            changed = False
            for inst in insts:
                si = inst.sync_info
                if si is not None and si.on_wait is not None and len(si.on_wait) > 1:
                    waits = list(si.on_wait)
                    for w in waits[:-1]:
                        ctr += 1
                        out.append(mybir.InstEventSemaphore(
                            name=f"wsplit_{ctr}",
                            engine=inst.engine,
                            ins=[], outs=[],
                            sync_info=mybir.SyncInfo(on_wait=[w], on_update=[]),
                            bass_nofuse=True,
                        ))
                    inst.sync_info = mybir.SyncInfo(
                        on_wait=[waits[-1]], on_update=list(si.on_update or []))
                    changed = True
                out.append(inst)
            if changed:
                blk.instructions.clear()
                for i2 in out:
                    blk.add_instruction(i2)


def _chunks(n):
    """Split even-length [0, n) into even-sized PSUM-bank chunks (<=512 each)."""
    assert n % 2 == 0
    if n <= 512:
        return [(0, n)]
    k = -(-n // 512)
    sizes = [(n // k) & ~1] * k
    rem, j = n - sum(sizes), 0
    while rem > 0:
        sizes[j] += 2
        rem -= 2
        j = (j + 1) % k
    out, s = [], 0
    for sz in sizes:
        out.append((s, sz))
        s += sz
    return out


def _build_nc(C0, C1, rep=1, internal_out=False):
    """One SPMD program; per-core inputs differ only in data.

    Two expert slots per core with token capacities C0 >= C1 (experts are
    paired big-with-small on the host). Per expert: all 32 i-tiles of HT
    stay resident in SBUF (bf16), so the down-projection runs once per
    expert with no DRAM accumulation.
    """
    nc = bass.Bass(target_bir_lowering=False)
    xt_d = [
        nc.dram_tensor("xt0", [NDT, 128, C0], _BF16, kind="ExternalInput"),
        nc.dram_tensor("xt1", [NDT, 128, C1], _BF16, kind="ExternalInput"),
    ]
    wv1_d = nc.dram_tensor("wv1", [E_LOC, NIT, 128, 2 * NDT * 128], _BF16,
                           kind="ExternalInput")
    w2_d = nc.dram_tensor("w2", [E_LOC, NDT, 128, NIT * 128], _BF16,
                          kind="ExternalInput")
    okind = "Internal" if internal_out else "ExternalOutput"
    yt_d = [
        nc.dram_tensor("yt0", [NDT, 128, C0], _F32, kind=okind),
        nc.dram_tensor("yt1", [NDT, 128, C1], _F32, kind=okind),
    ]
    sync_d = (nc.dram_tensor("sync", [128, 2], _F32, kind="ExternalOutput")
              if internal_out else None)

    with tile.TileContext(nc) as tc:
        with (
            tc.tile_pool(name="xt", bufs=2) as xt_pool,
            tc.tile_pool(name="ht", bufs=1) as ht_pool,
            tc.tile_pool(name="wv", bufs=3) as wv_pool,
            tc.tile_pool(name="w2", bufs=3) as w2_pool,
            tc.tile_pool(name="hs", bufs=2) as hs_pool,
            tc.tile_pool(name="yo", bufs=4) as yo_pool,
            tc.tile_pool(name="psg", bufs=1, space="PSUM") as psg,
            tc.tile_pool(name="psy", bufs=2, space="PSUM") as psy,
        ):
            pools = (xt_pool, ht_pool, wv_pool, w2_pool, hs_pool, yo_pool,
                     psg, psy)
            for rp in range(rep):
                for el, C in enumerate((C0, C1)):
                    _emit_expert(nc, pools, xt_d[el], wv1_d, w2_d, yt_d[el],
                                 rp, el, C)
            if sync_d is not None:
                sy = yo_pool.tile([128, 2], _F32, tag="sync", name="sync_t")
                nc.vector.memset(sy[:], 0.0)
                nc.sync.dma_start(sync_d[:, :], sy[:])
    nc.finalize()
    _split_multi_waits(nc)
    return nc


def _emit_expert(nc, pools, xt_d, wv1_d, w2_d, yt_d, rp, el, C):
    (xt_pool, ht_pool, wv_pool, w2_pool, hs_pool, yo_pool, psg, psy) = pools
    ch = _chunks(C)
    tb = f"{rp}_{el}"
    # --- token tiles (resident for the whole expert; prefetched) ---
    xts = xt_pool.tile([128, NDT, C], _BF16, tag="xt", name=f"xt_{tb}")
    for db in range(0, NDT, 4):
        nc.sync.dma_start(
            xts[:, db:db + 4, :],
            xt_d[db:db + 4].rearrange("t p c -> p t c"))
    hts = ht_pool.tile([128, NIT, C], _BF16, tag="ht", name=f"ht_{tb}")
    # --- stage 1+2: HT[it] = silu(W1T.T@X) * (V1T.T@X), all 32 i-tiles ---
    for it in range(NIT):
        wv = wv_pool.tile([128, 2, NDT, 128], _BF16, tag="wv",
                          name=f"wv_{tb}_{it}")
        nc.sync.dma_start(
            wv[:], wv1_d[el, it].rearrange("p (w t i) -> p w t i",
                                           w=2, t=NDT))
        gs = [psg.tile([128, cn], _F32, tag=f"g{ci}", name=f"g{ci}_{tb}_{it}")
              for ci, (c0, cn) in enumerate(ch)]
        for dt in range(NDT):
            for ci, (c0, cn) in enumerate(ch):
                nc.tensor.matmul(
                    gs[ci][:], wv[:, 0, dt, :], xts[:, dt, c0:c0 + cn],
                    start=(dt == 0), stop=(dt == NDT - 1))
        hss = []
        for ci, (c0, cn) in enumerate(ch):
            h1 = hs_pool.tile([128, cn], _BF16, tag=f"hs{ci}",
                              name=f"hs{ci}_{tb}_{it}")
            nc.scalar.activation(h1[:], gs[ci][:],
                                 mybir.ActivationFunctionType.Silu)
            hss.append(h1)
        us = [psg.tile([128, cn], _F32, tag=f"u{ci}", name=f"u{ci}_{tb}_{it}")
              for ci, (c0, cn) in enumerate(ch)]
        for dt in range(NDT):
            for ci, (c0, cn) in enumerate(ch):
                nc.tensor.matmul(
                    us[ci][:], wv[:, 1, dt, :], xts[:, dt, c0:c0 + cn],
                    start=(dt == 0), stop=(dt == NDT - 1))
        for ci, (c0, cn) in enumerate(ch):
            nc.vector.tensor_tensor(
                out=hts[:, it, c0:c0 + cn], in0=us[ci][:], in1=hss[ci][:],
                op=mybir.AluOpType.mult)
    # --- stage 3: YT[dt] = W2T.T @ HT over all 32 i-tiles ---
    w2_re = w2_d[el].rearrange("t p (u i) -> t p u i", u=NIT)
    for dt in range(NDT):
        w2t = w2_pool.tile([128, NIT, 128], _BF16, tag="w2",
                           name=f"w2_{tb}_{dt}")
        # w2 loads ride the ACT HWDGE ring so they never queue behind the
        # (pool-throttled) wv stream on the SP ring at stage boundaries.
        nc.scalar.dma_start(w2t[:], w2_re[dt])
        for ci, (c0, cn) in enumerate(ch):
            yt = psy.tile([128, cn], _F32, tag="y", name=f"y{ci}_{tb}_{dt}")
            for itl in range(NIT):
                nc.tensor.matmul(
                    yt[:], w2t[:, itl, :], hts[:, itl, c0:c0 + cn],
                    start=(itl == 0), stop=(itl == NIT - 1))
            yo = yo_pool.tile([128, cn], _F32, tag="yo",
                              name=f"yo{ci}_{tb}_{dt}")
            nc.scalar.activation(yo[:], yt[:],
                                 mybir.ActivationFunctionType.Copy)
            # y writes go out on the SWDGE ring, off both weight streams.
            nc.gpsimd.dma_start(yt_d[dt, :, c0:c0 + cn], yo[:])


def _prepare(hidden_states, router_w, ws, w2s, rep=1, internal_out=False):
    hs = np.ascontiguousarray(hidden_states, dtype=np.float32)
    rw = np.ascontiguousarray(router_w, dtype=np.float32)
    ws = np.asarray(ws, dtype=np.float32)
    w2s = np.asarray(w2s, dtype=np.float32)
    T, D_ = hs.shape
    assert (D_, ws.shape[0], ws.shape[1], w2s.shape[1], w2s.shape[2]) == \
        (D, E, 2 * I, D, I), "kernel compiled for DBRX 16x(2048->4096) shapes"

    # ---- routing on host (softmax -> top-4 -> renormalize) ----
    logits = hs @ rw.T                                   # [T, E]
    m = logits.max(axis=-1, keepdims=True)
    p = np.exp(logits - m)
    p /= p.sum(axis=-1, keepdims=True)
    topk_idx = np.argpartition(-p, TOP_K - 1, axis=-1)[:, :TOP_K]   # [T, 4]
    topk_val = np.take_along_axis(p, topk_idx, axis=-1)
    gates_w = topk_val / topk_val.sum(axis=-1, keepdims=True)

    tok_idx, tok_gate = [None] * E, [None] * E
    flat_e = topk_idx.ravel()
    flat_g = gates_w.ravel()
    flat_t = np.repeat(np.arange(T), TOP_K)
    order = np.argsort(flat_e, kind="stable")
    se, st, sg = flat_e[order], flat_t[order], flat_g[order]
    bounds = np.searchsorted(se, np.arange(E + 1))
    for e in range(E):
        tok_idx[e] = st[bounds[e]:bounds[e + 1]]
        tok_gate[e] = sg[bounds[e]:bounds[e + 1]]

    # ---- pair experts big-with-small across cores ----
    counts = np.array([len(t) for t in tok_idx])
    desc = np.argsort(-counts, kind="stable")
    # core c runs experts (desc[c], desc[15-c]); slot capacities:
    slot_expert = [[int(desc[c]), int(desc[2 * N_CORES - 1 - c])]
                   for c in range(N_CORES)]
    Cs = []
    for sl in range(E_LOC):
        Cmax = max(4, int(max(counts[slot_expert[c][sl]]
                              for c in range(N_CORES))))
        Cs.append(Cmax + (Cmax % 2))
    C0, C1 = Cs

    hs_bf = hs.astype(_NPBF16)
    ws_bf = ws.astype(_NPBF16)
    w2s_bf = w2s.astype(_NPBF16)

    # ---- build per-core inputs ----
    in_maps = []
    for c in range(N_CORES):
        im = {}
        wv1 = np.empty((E_LOC, NIT, 128, 2 * NDT * 128), dtype=_NPBF16)
        w2 = np.empty((E_LOC, NDT, 128, NIT * 128), dtype=_NPBF16)
        for el, Ce in enumerate((C0, C1)):
            e = slot_expert[c][el]
            xt = np.zeros((NDT, 128, Ce), dtype=_NPBF16)
            xg = hs_bf[tok_idx[e]]                       # [n_e, D]
            # xt[dt, p, c] = xg[c, dt*128+p]
            xt[:, :, :len(tok_idx[e])] = xg.T.reshape(NDT, 128, -1)
            im[f"xt{el}"] = xt
            w1 = ws_bf[e, :I, :]                         # [I, D]
            v1 = ws_bf[e, I:, :]
            # wv1[el, it, p, (w, dt, ii)] = {w1,v1}[it*128+ii, dt*128+p]
            wv = np.stack([w1, v1]).reshape(2, NIT, 128, NDT, 128)
            wv = wv.transpose(1, 4, 0, 3, 2)             # [it, p, w, dt, ii]
            wv1[el] = np.ascontiguousarray(wv).reshape(NIT, 128, -1)
            # w2[el, dt, p, (it, ii)] = w2s[e, dt*128+ii, it*128+p]
            w2e = w2s_bf[e].reshape(NDT, 128, NIT, 128)  # [dt, ii, it, p]
            w2[el] = np.ascontiguousarray(
                w2e.transpose(0, 3, 2, 1)).reshape(NDT, 128, -1)
        im["wv1"] = wv1
        im["w2"] = w2
        in_maps.append(im)

    def combine(results):
        out = np.zeros((T, D), dtype=np.float32)
        for c in range(N_CORES):
            for el in range(E_LOC):
                e = slot_expert[c][el]
                n_e = len(tok_idx[e])
                if n_e == 0:
                    continue
                yt = results[c][f"yt{el}"]               # [NDT, 128, C]
                y = yt.reshape(D, -1)[:, :n_e].T         # [n_e, D]
                out[tok_idx[e]] += tok_gate[e][:, None].astype(np.float32) * y
        return out

    nc = _build_nc(C0, C1, rep=rep, internal_out=internal_out)
    return {"nc": nc, "in_maps": in_maps, "combine": combine, "C": (C0, C1)}


def kernel(hidden_states, router_w, ws, w2s):
    from concourse.bass_utils import run_bass_kernel_spmd
    prep = _prepare(hidden_states, router_w, ws, w2s)
    res = run_bass_kernel_spmd(prep["nc"], prep["in_maps"],
                               core_ids=list(range(N_CORES)))
    return prep["combine"](res.results)
